# revision 1
# baseline (speedup 1.0000x reference)
"""AttentivePooling Trainium2 kernel.

Reference semantics (h_all: [T, B, D] f32, xin unused):
    h_last = h_all[-1]                       # [B, D]
    a[b, t] = <h_all[t, b, :], h_last[b, :]> / sqrt(D)
    r = relu(a)
    w = r / (sum_t r + 1e-9)
    out[b, d] = sum_t w[b, t] * h_all[t, b, d]

Strategy: data-parallel over B across 8 cores (8 batches/core, no
collectives).  Per batch on-device (pipelined two batches deep):
  - two 2MB SWDGE DMAs load h_b as 16 SBUF chunks [128(t), 512(d)]
    (t = c*128 + p).  (A single HWDGE dma_start with 2048 descriptors
    wedges the exec unit; SWDGE handles it.)
  - h_last[b] is broadcast across the 128 partitions with a
    partition-stride-0 DMA; ACT mirrors it into PSUM so the DVE
    multiplies read it through the PSUM port (halves SBUF read-port
    pressure; fp32 tensor_tensor is otherwise port-bound at 1x).
  - scores: elementwise multiply split DVE (11 chunks) / GPSIMD (5),
    free-dim reduction split ACT activation-accum (11, with the
    1/sqrt(D) scale folded in) / DVE tensor_reduce (5).  (The fused
    DVE tensor_tensor_reduce crashes the exec unit on this HW.)
  - ACT relu with accum_out produces weights + their per-partition sums
  - PE accumulates sum_t w_t * h_t into PSUM [1, 512] with float32r
    matmuls (w stationary): 1 cycle/row vs fp32's 4, at ~1e-4 rounding
  - PE reduces the weight-sum across partitions via a ones column
  - DVE computes 1/(Z + 1e-9); ACT scales the pooled vector
"""

import numpy as np
from contextlib import ExitStack

import concourse.bass as bass
import concourse.tile as tile
from concourse import bacc, mybir
from concourse.bass_utils import run_bass_kernel_spmd

T, B, D = 2048, 64, 512
NCORES = 8
BPC = B // NCORES  # batches per core
P = 128
TC = T // P  # 16 T-chunks per batch
SCALE = float(1.0 / np.sqrt(np.float32(D)))
DVE_REDUCE_CHUNKS = frozenset({3, 6, 9, 12, 15})  # reduces on DVE; rest on ACT
GP_MULT_CHUNKS = frozenset({2, 4, 7, 9, 12, 14})  # multiplies on GPSIMD; rest on DVE

_nc_cache = None


def _build():
    global _nc_cache
    if _nc_cache is not None:
        return _nc_cache
    nc = bacc.Bacc("TRN2", debug=False, target_bir_lowering=False, num_devices=NCORES)
    h = nc.dram_tensor("h", [T, BPC, D], mybir.dt.float32r, kind="ExternalInput")
    out = nc.dram_tensor("out", [BPC, D], mybir.dt.float32, kind="ExternalOutput")
    h_ap = h.ap()
    out_ap = out.ap()
    f32 = mybir.dt.float32
    f32r = mybir.dt.float32r

    with tile.TileContext(nc) as tc:
        with ExitStack() as ctx:
            hpool = ctx.enter_context(tc.tile_pool(name="h", bufs=5))
            psbcp = ctx.enter_context(tc.tile_pool(name="psb", bufs=2, space="PSUM"))
            tmpp = ctx.enter_context(tc.tile_pool(name="tmp", bufs=6))
            smallp = ctx.enter_context(tc.tile_pool(name="small", bufs=3))
            constp = ctx.enter_context(tc.tile_pool(name="const", bufs=1))
            psoutp = ctx.enter_context(tc.tile_pool(name="pso", bufs=3, space="PSUM"))
            pszp = ctx.enter_context(tc.tile_pool(name="psz", bufs=3, space="PSUM"))

            ones_col = constp.tile([P, 1], f32)
            nc.vector.memset(ones_col[:], 1.0)
            eps_tile = constp.tile([1, 1], f32)
            nc.vector.memset(eps_tile[:], 1e-9)

            def bcast(b):
                # broadcast h_last[b] to all 128 partitions via
                # partition-stride-0 DMA reads straight from DRAM.
                # One copy lands in PSUM (DVE reads it via the PSUM port,
                # halving SBUF read-port pressure), one in SBUF (GPSIMD
                # cannot read PSUM).
                src_bc = h_ap[T - 1 : T, b, :].bitcast(f32).broadcast_to([P, D])
                hlb = smallp.tile([P, D], f32, tag="hlb", name="hlb")
                nc.sync.dma_start(hlb[:], src_bc)
                psb = psbcp.tile([P, D], f32, tag="psb")
                nc.scalar.copy(psb[:], hlb[:])
                return psb, hlb

            HALF = TC // 2

            def load_h(b):
                t = hpool.tile([P, TC, D], f32r, tag="hsb", name="h_sb")
                src_ap = h_ap[:, b, :].rearrange("(c p) d -> p c d", p=P)
                nc.gpsimd.dma_start(t[:, 0:HALF, :], src_ap[:, 0:HALF, :])
                nc.gpsimd.dma_start(t[:, HALF:TC, :], src_ap[:, HALF:TC, :])
                return t

            h_tiles = {}
            for b in range(min(2, BPC)):
                h_tiles[b] = load_h(b)
            psb_tiles = {0: bcast(0)}

            for b in range(BPC):
                h_sb = h_tiles.pop(b)
                psb, hlb = psb_tiles.pop(b)

                # scores: scr[p, c] = sum_d h[t, d] * hl[d] * SCALE
                scr = smallp.tile([P, TC], f32, tag="scr")
                for c in range(TC):
                    tmp = tmpp.tile([P, D], f32, tag="tmp")
                    if c in GP_MULT_CHUNKS:
                        nc.gpsimd.tensor_tensor(
                            tmp[:],
                            h_sb[:, c, :].bitcast(f32),
                            hlb[:],
                            mybir.AluOpType.mult,
                        )
                    else:
                        nc.vector.tensor_tensor(
                            tmp[:],
                            h_sb[:, c, :].bitcast(f32),
                            psb[:],
                            mybir.AluOpType.mult,
                        )
                    if c in DVE_REDUCE_CHUNKS:
                        nc.vector.tensor_reduce(
                            scr[:, c : c + 1],
                            tmp[:],
                            mybir.AxisListType.X,
                            mybir.AluOpType.add,
                        )
                    else:
                        nc.scalar.activation(
                            tmp[:],
                            tmp[:],
                            mybir.ActivationFunctionType.Copy,
                            scale=SCALE,
                            accum_out=scr[:, c : c + 1],
                        )

                # rescale the DVE-reduced columns (ACT ones had SCALE folded)
                for c in sorted(DVE_REDUCE_CHUNKS):
                    nc.vector.tensor_scalar_mul(
                        scr[:, c : c + 1], scr[:, c : c + 1], SCALE
                    )

                # relu + per-partition sum of relu'd scores
                w = smallp.tile([P, TC], f32r, tag="w")
                zcol = smallp.tile([P, 1], f32, tag="z")
                nc.scalar.activation(
                    w[:], scr[:], mybir.ActivationFunctionType.Relu, accum_out=zcol[:]
                )

                # next batch's broadcast goes to PE BEFORE this batch's
                # pooling burst, so the next scores phase is not blocked
                # behind the pooling in PE program order
                if b + 1 < BPC:
                    psb_tiles[b + 1] = bcast(b + 1)
                if b + 2 < BPC:
                    h_tiles[b + 2] = load_h(b + 2)

                # pooled[d] = sum_t w_t * h[t, d] accumulated over chunks
                pout = psoutp.tile([1, D], f32)
                for c in range(TC):
                    nc.tensor.matmul(
                        pout[:],
                        w[:, c : c + 1],
                        h_sb[:, c, :],
                        start=(c == 0),
                        stop=(c == TC - 1),
                    )
                # Z = sum over all t of relu'd scores
                pz = pszp.tile([1, 1], f32)
                nc.tensor.matmul(pz[:], zcol[:], ones_col[:], start=True, stop=True)

                zeps = smallp.tile([1, 1], f32, tag="zeps")
                nc.scalar.activation(
                    zeps[:],
                    pz[:],
                    mybir.ActivationFunctionType.Identity,
                    bias=eps_tile[0:1, 0:1],
                )
                zrec = smallp.tile([1, 1], f32, tag="zrec")
                nc.vector.reciprocal(zrec[:], zeps[:])
                res = smallp.tile([1, D], f32, tag="res")
                nc.scalar.mul(res[:], pout[:], zrec[0:1, 0:1])
                nc.sync.dma_start(out_ap[b : b + 1, :], res[:])

    nc.finalize()
    _nc_cache = nc
    return nc


def _run(h_all: np.ndarray, trace: bool = False):
    nc = _build()
    h_all = np.ascontiguousarray(np.asarray(h_all), dtype=np.float32)
    assert h_all.shape == (T, B, D)
    in_maps = [
        {"h": np.ascontiguousarray(h_all[:, c * BPC : (c + 1) * BPC, :])}
        for c in range(NCORES)
    ]
    r = run_bass_kernel_spmd(nc, in_maps, list(range(NCORES)), trace=trace)
    out = np.concatenate([r.results[c]["out"] for c in range(NCORES)], axis=0)
    return out, r


def kernel(h_all: np.ndarray, xin: np.ndarray | None = None) -> np.ndarray:
    out, _ = _run(h_all)
    return out



# revision 21
# speedup vs baseline: 1.0109x; 1.0109x over previous
"""AttentivePooling Trainium2 kernel (chunk-major streaming, bf16).

Reference semantics (h_all: [T, B, D] f32, xin unused):
    h_last = h_all[-1]                       # [B, D]
    a[b, t] = <h_all[t, b, :], h_last[b, :]> / sqrt(D)
    r = relu(a)
    w = r / (sum_t r + 1e-9)
    out[b, d] = sum_t w[b, t] * h_all[t, b, d]

The 1/sqrt(D) scale cancels in w (relu is positively homogeneous) and
the eps is negligible: a[T-1] = |h_last|^2/sqrt(D) > 0 always, so the
relu-sum is >> 1e-9.  We pool with unnormalized relu scores and divide
by their sum at the end.

Layout: data-parallel over B across 8 cores (8 batches/core).  Per
core the shard h[:, 0:8, :] is streamed t-chunk-major: chunk c is the
fully contiguous DRAM block h[128c:128c+128, :, :] -> one 2MB SWDGE
DMA (f32 read, inline cast to bf16) into an SBUF tile
[128(t), 4096(b*d)].  HBM traffic is unchanged (f32 reads); bf16
on-chip doubles DVE throughput and, critically, lets the pooling
matmuls target PSUM partition quadrants (f32r matmuls may only write
partition 0; bf16 may write 0/32/64 - walrus s3d3_mm_valid_dst_partition).
Per chunk:
  - scores: tmp = h_c * hlb (hlb = h_last broadcast to 128 partitions,
    built once at startup via PE ones-outer-products + PSUM->SBUF
    casts); multiply split DVE (b 0-5, two ops) / GPSIMD (b 6-7);
    d-reduce split ACT activation-accum (b 0-5, f32 accum) / DVE
    segmented tensor_reduce (b 6-7, f32 out)
  - relu -> w_c [128, 8] bf16 (unnormalized weights)
  - PE (bf16): per b accumulate w_c[:,b]^T @ h_c[:,b-block] into PSUM
    row (b%3)*32 of tile b//3 (3 [96,512] tiles hold 8 rows in 3
    banks); plus w_c^T @ ones -> zrow [8,1] f32 (relu-score sums)
Epilogue: scatter zrow to the same quadrant rows via an f32r 0/1
selector matmul (keeps the reciprocal and the per-partition scale
lane-aligned with pout), zrec = 1/z (DVE), res = pout * zrec (one ACT
per tile over all 96 partitions; junk rows unused), 3 partition-strided
2KB stores.  All normalization math stays f32.
"""

import numpy as np
from contextlib import ExitStack

import concourse.bass as bass
import concourse.tile as tile
from concourse import bacc, mybir
from concourse.bass_utils import run_bass_kernel_spmd

T, B, D = 2048, 64, 512
NCORES = 8
BPC = B // NCORES  # batches per core
P = 128
NCHUNK = T // P  # 16
FB = BPC * D  # 4096 free elems per partition per chunk
DVE_B = 6  # batches 0..5 multiplied on DVE (reduced on ACT)
GP_B = BPC - DVE_B  # batches 6..7 multiplied on GPSIMD (reduced on DVE)
PREFETCH = 6  # chunks in flight

_nc_cache = None


def _build():
    global _nc_cache
    if _nc_cache is not None:
        return _nc_cache
    nc = bacc.Bacc("TRN2", debug=False, target_bir_lowering=False, num_devices=NCORES)
    h = nc.dram_tensor("h", [T, BPC, D], mybir.dt.float32, kind="ExternalInput")
    # selt[t][b, (b%3)*32] = 1 for b//3 == t: scatters zrow to the
    # quadrant rows used by the pooling matmuls (engine ops cannot
    # address partition bases other than 0/32/64/96, so this constant
    # comes from the host instead of per-element memsets)
    selt = nc.dram_tensor("selt", [P, 3 * 96], mybir.dt.float32, kind="ExternalInput")
    out = nc.dram_tensor("out", [BPC, D], mybir.dt.float32, kind="ExternalOutput")
    h_ap = h.ap()
    out_ap = out.ap()
    f32 = mybir.dt.float32
    f32r = mybir.dt.float32r
    bf16 = mybir.dt.bfloat16

    def quad(b):  # pooling row for batch b: PSUM tile b//3, partition (b%3)*32
        return b // 3, (b % 3) * 32

    with tile.TileContext(nc) as tc:
        with ExitStack() as ctx:
            hpool = ctx.enter_context(tc.tile_pool(name="h", bufs=PREFETCH + 1))
            tmpap = ctx.enter_context(tc.tile_pool(name="tmpa", bufs=2))
            tmpbp = ctx.enter_context(tc.tile_pool(name="tmpb", bufs=2))
            scrp = ctx.enter_context(tc.tile_pool(name="scr", bufs=3))
            constp = ctx.enter_context(tc.tile_pool(name="const", bufs=1))
            resp = ctx.enter_context(tc.tile_pool(name="res", bufs=3))
            psbcp = ctx.enter_context(tc.tile_pool(name="psbc", bufs=2, space="PSUM"))
            psoutp = ctx.enter_context(tc.tile_pool(name="pso", bufs=1, space="PSUM"))
            pszp = ctx.enter_context(tc.tile_pool(name="psz", bufs=1, space="PSUM"))
            zpsp = ctx.enter_context(tc.tile_pool(name="zps", bufs=1, space="PSUM"))

            ones_f = constp.tile([P, 1], f32)
            nc.vector.memset(ones_f[:], 1.0)
            ones_row_f = constp.tile([1, P], f32)
            nc.vector.memset(ones_row_f[:], 1.0)
            ones_row = constp.tile([1, P], bf16)
            nc.vector.tensor_copy(ones_row[:], ones_row_f[:])
            ones_col = constp.tile([P, 1], bf16)
            nc.vector.tensor_copy(ones_col[:], ones_f[:])

            # selector matrices (from host): sel[t] = sel_sb[:, 96t:96t+96]
            # (plain f32 matmul: f32r requires rhs free size >= 2 and the
            # scatter streams a single column; rows >= BPC are zero)
            sel_sb = constp.tile([P, 3 * 96], f32)
            nc.sync.dma_start(sel_sb[:], selt.ap())

            # h_last = h[T-1, :, :] -> one partition (cast to bf16), then
            # PE-broadcast to all 128 partitions, PSUM -> SBUF bf16.
            hl1 = constp.tile([1, FB], bf16)
            nc.gpsimd.dma_start(
                hl1[:], h_ap[T - 1 : T, :, :].rearrange("t b d -> t (b d)")
            )
            hlb = constp.tile([P, FB], bf16)
            for b in range(BPC):
                pbc = psbcp.tile([P, D], f32, tag="pbc")
                nc.tensor.matmul(
                    pbc[:],
                    ones_row[:],
                    hl1[0:1, b * D : (b + 1) * D],
                    start=True,
                    stop=True,
                )
                if b % 2 == 0:
                    nc.scalar.copy(hlb[:, b * D : (b + 1) * D], pbc[:])
                else:
                    nc.vector.tensor_copy(hlb[:, b * D : (b + 1) * D], pbc[:])

            pouts = [
                psoutp.tile([96, D], f32, tag=f"pout{t}", name=f"pout{t}")
                for t in range(3)
            ]
            zrow = pszp.tile([BPC, 1], f32)

            def load(c):
                t_ = hpool.tile([P, FB], bf16, tag="hsb", name="h_sb")
                nc.gpsimd.dma_start(
                    t_[:],
                    h_ap[c * P : (c + 1) * P, :, :].rearrange("t b d -> t (b d)"),
                )
                return t_

            h_tiles = {}
            for c in range(min(PREFETCH, NCHUNK)):
                h_tiles[c] = load(c)

            for c in range(NCHUNK):
                h_sb = h_tiles.pop(c)
                first = c == 0
                last = c == NCHUNK - 1

                # elementwise h * h_last_broadcast (bf16, DVE 2x mode)
                tmpa = tmpap.tile([P, DVE_B * D], bf16, tag="tmpa")
                tmpb = tmpbp.tile([P, GP_B * D], bf16, tag="tmpb")
                nc.gpsimd.tensor_tensor(
                    tmpb[:],
                    h_sb[:, DVE_B * D : FB],
                    hlb[:, DVE_B * D : FB],
                    mybir.AluOpType.mult,
                )
                # DVE's half in two ops so ACT reductions start earlier
                H2 = DVE_B // 2
                nc.vector.tensor_tensor(
                    tmpa[:, 0 : H2 * D],
                    h_sb[:, 0 : H2 * D],
                    hlb[:, 0 : H2 * D],
                    mybir.AluOpType.mult,
                )
                nc.vector.tensor_tensor(
                    tmpa[:, H2 * D : DVE_B * D],
                    h_sb[:, H2 * D : DVE_B * D],
                    hlb[:, H2 * D : DVE_B * D],
                    mybir.AluOpType.mult,
                )

                # reduce over d: scr[p, b] = sum_d tmp[p, b*D+d]  (f32 accum)
                scr = scrp.tile([P, BPC], f32, tag="scr")
                for b in range(DVE_B):
                    nc.scalar.activation(
                        tmpa[:, b * D : (b + 1) * D],
                        tmpa[:, b * D : (b + 1) * D],
                        mybir.ActivationFunctionType.Copy,
                        accum_out=scr[:, b : b + 1],
                    )
                nc.vector.tensor_reduce(
                    scr[:, DVE_B:BPC],
                    tmpb[:].rearrange("p (b d) -> p b d", b=GP_B),
                    mybir.AxisListType.X,
                    mybir.AluOpType.add,
                )

                # relu -> unnormalized weights (bf16)
                w_c = scrp.tile([P, BPC], bf16, tag="w")
                nc.scalar.activation(w_c[:], scr[:], mybir.ActivationFunctionType.Relu)

                # pooling: pout row of b += w_c[:,b]^T @ h_c[:,b-block]
                for b in range(BPC):
                    t, q = quad(b)
                    nc.tensor.matmul(
                        pouts[t][q : q + 1, :],
                        w_c[:, b : b + 1],
                        h_sb[:, b * D : (b + 1) * D],
                        start=first,
                        stop=last,
                    )
                nc.tensor.matmul(zrow[:], w_c[:], ones_col[:], start=first, stop=last)

                if c + PREFETCH < NCHUNK:
                    h_tiles[c + PREFETCH] = load(c + PREFETCH)

            # epilogue: out = pout / z  (eps negligible, see header)
            zrow_f = scrp.tile([P, 1], f32, tag="zf")
            nc.vector.memset(zrow_f[:], 0.0)
            nc.vector.tensor_copy(zrow_f[0:BPC, :], zrow[:])
            for t in range(3):
                zps = zpsp.tile([96, 1], f32, tag="zps")
                nc.tensor.matmul(
                    zps[:],
                    sel_sb[:, 96 * t : 96 * (t + 1)],
                    zrow_f[:],
                    start=True,
                    stop=True,
                )
                zrec = scrp.tile([96, 1], f32, tag=f"zrec{t}")
                nc.vector.reciprocal(zrec[:], zps[:])
                res = resp.tile([96, D], f32, tag=f"res{t}", name="res")
                nc.scalar.activation(
                    res[:],
                    pouts[t][:],
                    mybir.ActivationFunctionType.Copy,
                    scale=zrec[:, 0:1],
                )
                nb = min(BPC - 3 * t, 3)  # rows used in this tile (3, 3, 2)
                src = res[:].rearrange("(g r) d -> g r d", r=32)[0:nb, 0, :]
                nc.sync.dma_start(out_ap[3 * t : 3 * t + nb, :], src)

    nc.finalize()
    _nc_cache = nc
    return nc


def _run(h_all: np.ndarray, trace: bool = False):
    nc = _build()
    h_all = np.ascontiguousarray(np.asarray(h_all), dtype=np.float32)
    assert h_all.shape == (T, B, D)
    sel_np = np.zeros((P, 3 * 96), dtype=np.float32)
    for b in range(BPC):
        sel_np[b, 96 * (b // 3) + (b % 3) * 32] = 1.0
    in_maps = [
        {
            "h": np.ascontiguousarray(h_all[:, c * BPC : (c + 1) * BPC, :]),
            "selt": sel_np,
        }
        for c in range(NCORES)
    ]
    r = run_bass_kernel_spmd(nc, in_maps, list(range(NCORES)), trace=trace)
    out = np.concatenate([r.results[c]["out"] for c in range(NCORES)], axis=0)
    return out, r


def kernel(h_all: np.ndarray, xin: np.ndarray | None = None) -> np.ndarray:
    out, _ = _run(h_all)
    return out
